# revision 1
# baseline (speedup 1.0000x reference)
"""MultiHeadDiffAttention kernel for 8 trn2 NeuronCores.

Sharding: tensor-parallel over heads (H=8, one head per core).
Per core (head h), per batch:
  qT/kT = Wq_h @ x.T   [128 feat, 2048 tok]  (bf16 matmuls, f32 accum)
  v     = x @ Wv_h.T   [2048 tok, 128 dh]
  scoresT[k, q] per diff-branch via row-packed PE matmuls (c=64, concurrent)
  exp on ScalarE, both branches in one [128,1024] ACTIVATE (scores ~ N(0,1):
  no max subtraction needed)
  denominators d[q] = sum_k exp via ones-stationary matmul (LDW is 27ns)
  uT = v.T-stationary matmul streaming exp at n=512 (no LDW bottleneck)
  r = 1/d broadcast across partitions via c=1 matmul with ones / (-dw) weights
  oT = u1T*R1 - dw*u2T*R2 (DVE)
o stays transposed [dh, tok]. Two AllToAlls (one per batch, unused half
zero-filled) redistribute head-sharded -> token-sharded; the first hides under
batch-1 compute. Post-A2A the joint-head RMS reduction uses the same
ones-matmul trick; norm_w and (1-dw) are folded into Wo on the host; Wo
consumes oT directly (no transposes). Host concatenates the 8 token slices.
"""

import os
import sys

import numpy as np

if "/opt/trn_rl_repo" not in sys.path:
    sys.path.insert(0, "/opt/trn_rl_repo")

B, S, E, H = 2, 2048, 1024, 8
DH = E // H          # 128
F = DH // 2          # 64
P = 128              # partitions
NCORES = 8
TOK = B * S          # 4096
TPC = TOK // NCORES  # 512 tokens per core (phase-3 slice)
EC = E // P          # 8 e-chunks
KC = S // P          # 16 k-chunks per batch
QBS = 512            # q-block size
QB = S // QBS        # 4 q-blocks per batch
TT = TPC // P        # 4 token tiles for Wo lhsT
EPS = float(np.finfo(np.float32).eps)

LAST_RESULTS = None  # BassKernelResults of the most recent run (test.py reads this)

_NC_CACHE: dict = {}


def _build(dw: float):
    import concourse.bass as bass
    import concourse.mybir as mybir
    import concourse.tile as tile
    from concourse import bacc

    dt = mybir.dt
    AF = mybir.ActivationFunctionType

    nc = bacc.Bacc("TRN2", target_bir_lowering=False, debug=False, num_devices=NCORES)

    xT_d = nc.dram_tensor("xT", [B, E, S], dt.bfloat16, kind="ExternalInput")
    wqT_d = nc.dram_tensor("wqT", [E, DH], dt.bfloat16, kind="ExternalInput")
    wkT_d = nc.dram_tensor("wkT", [E, DH], dt.bfloat16, kind="ExternalInput")
    wvT_d = nc.dram_tensor("wvT", [E, DH], dt.bfloat16, kind="ExternalInput")
    woT_d = nc.dram_tensor("woT", [E, E], dt.bfloat16, kind="ExternalInput")
    out_d = nc.dram_tensor("out", [TPC, E], dt.float32, kind="ExternalOutput")

    with tile.TileContext(nc) as tc:
        with (
            tc.tile_pool(name="consts", bufs=1) as consts,
            tc.tile_pool(name="xt", bufs=1) as xtp,
            tc.tile_pool(name="qk", bufs=2) as qkp,
            tc.tile_pool(name="vp", bufs=2) as vp,
            tc.tile_pool(name="expp", bufs=6) as expp,
            tc.tile_pool(name="osb", bufs=2) as osb,
            tc.tile_pool(name="small", bufs=2) as small,
            tc.tile_pool(name="mid", bufs=2) as mid,
            tc.tile_pool(name="p3", bufs=1) as p3,
            tc.tile_pool(name="dram", bufs=1, space="DRAM") as dram,
            tc.tile_pool(name="psA", bufs=2, space="PSUM") as psA,
            tc.tile_pool(name="psU", bufs=2, space="PSUM") as psU,
            tc.tile_pool(name="psS", bufs=2, space="PSUM") as psS,
        ):
            eps_t = consts.tile([P, 1], dt.float32, tag="eps")
            nc.vector.memset(eps_t, EPS)
            ones_col = consts.tile([P, 32], dt.bfloat16, tag="ones_col")
            nc.vector.memset(ones_col, 1.0)
            ones_c1 = consts.tile([1, P], dt.float32, tag="ones_c1")
            nc.vector.memset(ones_c1, 1.0)
            negdw_c1 = consts.tile([1, P], dt.float32, tag="negdw_c1")
            nc.vector.memset(negdw_c1, -dw)
            zero_t = consts.tile([P, QBS], dt.bfloat16, tag="zero_t")
            nc.vector.memset(zero_t, 0.0)

            wq_sb = consts.tile([P, EC, DH], dt.bfloat16, tag="wq")
            wk_sb = consts.tile([P, EC, DH], dt.bfloat16, tag="wk")
            wv_sb = consts.tile([P, EC, DH], dt.bfloat16, tag="wv")
            for w_sb, w_d in ((wq_sb, wqT_d), (wk_sb, wkT_d), (wv_sb, wvT_d)):
                nc.sync.dma_start(
                    out=w_sb, in_=w_d.rearrange("(c p) d -> p c d", p=P)
                )
            wo_sb = consts.tile([P, EC, E], dt.bfloat16, tag="wo")
            nc.sync.dma_start(out=wo_sb, in_=woT_d.rearrange("(c p) e -> p c e", p=P))

            # Per-batch AllToAll bounce buffers: [dst block, dh, tok-in-block].
            a2a_in = [
                dram.tile([NCORES, DH, TPC], dt.bfloat16, tag=f"a2a_in{b}", name=f"a2a_in{b}")
                for b in range(B)
            ]
            a2a_out = [
                dram.tile([NCORES, DH, TPC], dt.bfloat16, tag=f"a2a_out{b}", name=f"a2a_out{b}")
                for b in range(B)
            ]
            # zero-fill the half of each input that carries no real data
            for b in range(B):
                for d in range(NCORES):
                    if d // QB != b:
                        nc.sync.dma_start(out=a2a_in[b][d], in_=zero_t)

            for b in range(B):
                xt = xtp.tile([P, EC, S], dt.bfloat16, tag="xt")
                xT_v = xT_d[b].rearrange("(c p) t -> c p t", p=P)
                for ec in range(EC):
                    nc.sync.dma_start(out=xt[:, ec, :], in_=xT_v[ec])

                # --- projections (q/k/v groups interleaved so the 2 PSUM
                # slots get ~2 group-times before reuse) ---
                qT = qkp.tile([P, S], dt.bfloat16, tag="qT")
                kT = qkp.tile([P, S], dt.bfloat16, tag="kT")
                v = vp.tile([P, KC, DH], dt.bfloat16, tag="v")

                def qk_group(w_sb, dst, tb):
                    ps = psA.tile([P, 2, QBS], dt.float32, tag="sc", name="ps")
                    for ec in range(EC):
                        nc.tensor.matmul(
                            ps[:, 0, :],
                            lhsT=w_sb[:, ec, :],
                            rhs=xt[:, ec, tb * 512 : (tb + 1) * 512],
                            start=(ec == 0),
                            stop=(ec == EC - 1),
                        )
                    nc.vector.tensor_copy(dst[:, tb * 512 : (tb + 1) * 512], ps[:, 0, :])

                def v_group(kt):
                    ps = psA.tile([P, 2, QBS], dt.float32, tag="sc", name="ps")
                    for ec in range(EC):
                        nc.tensor.matmul(
                            ps[:, 0, :DH],
                            lhsT=xt[:, ec, kt * P : (kt + 1) * P],
                            rhs=wv_sb[:, ec, :],
                            start=(ec == 0),
                            stop=(ec == EC - 1),
                        )
                    nc.vector.tensor_copy(v[:, kt, :], ps[:, 0, :DH])

                for tb in range(S // 512):
                    qk_group(wq_sb, qT, tb)
                    qk_group(wk_sb, kT, tb)
                    v_group(4 * tb)
                    v_group(4 * tb + 1)
                    v_group(4 * tb + 2)
                    v_group(4 * tb + 3)

                # --- attention ---
                for qb in range(QB):
                    qs = slice(qb * QBS, (qb + 1) * QBS)
                    # accumulators for this q-block
                    dsum1 = psS.tile([32, QBS], dt.float32, tag="dsum")
                    dsum2 = psS.tile([32, QBS], dt.float32, tag="dsum")
                    u1 = psU.tile([P, QBS], dt.float32, tag="u")
                    u2 = psU.tile([P, QBS], dt.float32, tag="u")
                    def consume(kt, ee):
                        # denominators + attention@V for k-chunk kt
                        nc.tensor.matmul(
                            dsum1, lhsT=ones_col, rhs=ee[:, 0, :],
                            start=(kt == 0), stop=(kt == KC - 1),
                        )
                        nc.tensor.matmul(
                            dsum2, lhsT=ones_col, rhs=ee[:, 1, :],
                            start=(kt == 0), stop=(kt == KC - 1),
                        )
                        nc.tensor.matmul(
                            u1, lhsT=v[:, kt, :], rhs=ee[:, 0, :],
                            start=(kt == 0), stop=(kt == KC - 1),
                        )
                        nc.tensor.matmul(
                            u2, lhsT=v[:, kt, :], rhs=ee[:, 1, :],
                            start=(kt == 0), stop=(kt == KC - 1),
                        )

                    prev = None
                    for kt in range(KC):
                        ks = slice(kt * P, (kt + 1) * P)
                        s12 = psA.tile([P, 2, QBS], dt.float32, tag="sc")
                        nc.tensor.matmul(s12[:, 0, :], lhsT=kT[0:F, ks], rhs=qT[0:F, qs])
                        nc.tensor.matmul(s12[:, 1, :], lhsT=kT[F:P, ks], rhs=qT[F:P, qs])
                        ee = expp.tile([P, 2, QBS], dt.bfloat16, tag="ee")
                        nc.scalar.activation(ee, s12, AF.Exp, scale=F**-0.5)
                        # consume the PREVIOUS k-chunk so the PE never waits on
                        # this chunk's exp (software pipeline, lag 1)
                        if prev is not None:
                            consume(*prev)
                        prev = (kt, ee)
                    consume(*prev)

                    rrow1 = small.tile([1, QBS], dt.float32, tag="rrow1")
                    rrow2 = small.tile([1, QBS], dt.float32, tag="rrow2")
                    nc.vector.reciprocal_approx_fast(rrow1, dsum1[0:1, :])
                    nc.vector.reciprocal_approx_fast(rrow2, dsum2[0:1, :])
                    # broadcast recips across partitions; fold -dw into branch 2
                    rps = psA.tile([P, 2, QBS], dt.float32, tag="sc")
                    nc.tensor.matmul(rps[:, 0, :], lhsT=ones_c1, rhs=rrow1)
                    nc.tensor.matmul(rps[:, 1, :], lhsT=negdw_c1, rhs=rrow2)
                    rr = mid.tile([P, 2, QBS], dt.float32, tag="rr")
                    nc.vector.tensor_copy(rr, rps)

                    t1 = mid.tile([P, QBS], dt.float32, tag="t1")
                    nc.vector.tensor_mul(t1, u1, rr[:, 0, :])
                    t2 = mid.tile([P, QBS], dt.float32, tag="t2")
                    nc.vector.tensor_mul(t2, u2, rr[:, 1, :])
                    oT = osb.tile([P, QBS], dt.bfloat16, tag="oT")
                    nc.vector.tensor_add(oT, t1, t2)
                    nc.sync.dma_start(out=a2a_in[b][b * QB + qb], in_=oT)

                nc.gpsimd.collective_compute(
                    "AllToAll",
                    mybir.AluOpType.bypass,
                    replica_groups=[list(range(NCORES))],
                    ins=[a2a_in[b].opt()],
                    outs=[a2a_out[b].opt()],
                )
                if b == 0:
                    # start phase-3 work that only needs the first half:
                    # load + square oT1 under batch-1 compute
                    oT1 = p3.tile([P, H, TPC], dt.bfloat16, tag="oT1")
                    nc.sync.dma_start(
                        out=oT1, in_=a2a_out[0].rearrange("h p t -> p h t")
                    )
                    sq1 = p3.tile([P, H, TPC], dt.bfloat16, tag="sq1")
                    nc.scalar.activation(sq1, oT1, AF.Square)

            # --- phase 3: RMS norm + output projection on my 512-token slice ---
            # a2a_out[b] block s = head s, [dh, my 512 tokens]; exactly one of
            # the two buffers holds real data on this rank, the other zeros.
            oT2 = p3.tile([P, H, TPC], dt.bfloat16, tag="oT2")
            nc.sync.dma_start(out=oT2, in_=a2a_out[1].rearrange("h p t -> p h t"))
            oTs = p3.tile([P, H, TPC], dt.bfloat16, tag="oTs")
            nc.vector.tensor_add(oTs, oT1, oT2)

            sq2 = p3.tile([P, H, TPC], dt.bfloat16, tag="sq2")
            nc.scalar.activation(sq2, oT2, AF.Square)
            ssq = psS.tile([32, TPC], dt.float32, tag="dsum")
            for fc in range(EC):
                nc.tensor.matmul(
                    ssq, lhsT=ones_col, rhs=sq1[:, fc, :],
                    start=(fc == 0), stop=False,
                )
            for fc in range(EC):
                nc.tensor.matmul(
                    ssq, lhsT=ones_col, rhs=sq2[:, fc, :],
                    start=False, stop=(fc == EC - 1),
                )
            sroot = small.tile([1, TPC], dt.float32, tag="sroot")
            nc.scalar.activation(
                sroot, ssq[0:1, :], AF.Sqrt, scale=1.0 / E, bias=eps_t[0:1, :]
            )
            rmsrow = small.tile([1, TPC], dt.float32, tag="rmsrow")
            nc.vector.reciprocal_approx_fast(rmsrow, sroot)
            rmsps = psA.tile([P, 2, QBS], dt.float32, tag="sc")
            nc.tensor.matmul(rmsps[:, 0, :], lhsT=ones_c1, rhs=rmsrow)
            rmsb = mid.tile([P, QBS], dt.float32, tag="rmsb")
            nc.vector.tensor_copy(rmsb, rmsps[:, 0, :])

            nrmT = p3.tile([P, H, TPC], dt.bfloat16, tag="nrmT")
            for fc in range(EC):
                nc.vector.tensor_mul(nrmT[:, fc, :], oTs[:, fc, :], rmsb)

            out_v = out_d.rearrange("(q p) e -> q p e", p=P)
            for tt in range(TT):
                out_sb = p3.tile([P, E], dt.float32, tag="out_sb", bufs=2)
                for nb in range(E // 512):
                    ps = psA.tile([P, 2, QBS], dt.float32, tag="sc")
                    for fc in range(EC):
                        nc.tensor.matmul(
                            ps[:, 0, :],
                            lhsT=nrmT[:, fc, tt * P : (tt + 1) * P],
                            rhs=wo_sb[:, fc, nb * 512 : (nb + 1) * 512],
                            start=(fc == 0),
                            stop=(fc == EC - 1),
                        )
                    nc.vector.tensor_copy(
                        out_sb[:, nb * 512 : (nb + 1) * 512], ps[:, 0, :]
                    )
                nc.sync.dma_start(out=out_v[tt], in_=out_sb)

    nc.compile()
    return nc


def _get_nc(dw: float):
    key = round(float(dw), 9)
    if key not in _NC_CACHE:
        _NC_CACHE[key] = _build(float(dw))
    return _NC_CACHE[key]


def kernel(x, Wq, Wk, Wv, norm_w, Wo, bo, diff_weight):
    import ml_dtypes

    from concourse.bass_utils import run_bass_kernel_spmd

    global LAST_RESULTS

    bf16 = ml_dtypes.bfloat16
    x = np.asarray(x, dtype=np.float32)
    Wq = np.asarray(Wq, dtype=np.float32)
    Wk = np.asarray(Wk, dtype=np.float32)
    Wv = np.asarray(Wv, dtype=np.float32)
    Wo = np.asarray(Wo, dtype=np.float32)
    norm_w = np.asarray(norm_w, dtype=np.float32)
    bo = np.asarray(bo, dtype=np.float32)
    dw = float(np.asarray(diff_weight))

    nc = _get_nc(dw)

    xT = np.ascontiguousarray(x.transpose(0, 2, 1)).astype(bf16)  # [B, E, S]
    woT = np.ascontiguousarray(
        (Wo * norm_w.reshape(-1)[None, :] * (1.0 - dw)).T
    ).astype(bf16)  # [E(feat), E(out)]

    in_maps = []
    for h in range(NCORES):
        rows = slice(h * DH, (h + 1) * DH)
        in_maps.append(
            {
                "xT": xT,
                "wqT": np.ascontiguousarray(Wq[rows, :].T).astype(bf16),
                "wkT": np.ascontiguousarray(Wk[rows, :].T).astype(bf16),
                "wvT": np.ascontiguousarray(Wv[rows, :].T).astype(bf16),
                "woT": woT,
            }
        )

    res = run_bass_kernel_spmd(
        nc,
        in_maps,
        core_ids=list(range(NCORES)),
        trace=bool(os.environ.get("KERNEL_TRACE")),
    )
    LAST_RESULTS = res

    full = np.concatenate([res.results[c]["out"] for c in range(NCORES)], axis=0)
    full = full + (1.0 - dw) * bo[None, :]
    return full.reshape(B, S, E).astype(np.float32)


if __name__ == "__main__":
    rng = np.random.default_rng(0)
    sc = E**-0.5
    ins = {
        "x": rng.standard_normal((B, S, E), dtype=np.float32),
        "Wq": rng.standard_normal((E, E), dtype=np.float32) * sc,
        "Wk": rng.standard_normal((E, E), dtype=np.float32) * sc,
        "Wv": rng.standard_normal((E, E), dtype=np.float32) * sc,
        "norm_w": np.ones((H, DH), dtype=np.float32),
        "Wo": rng.standard_normal((E, E), dtype=np.float32) * sc,
        "bo": np.zeros((E,), dtype=np.float32),
        "diff_weight": np.float32(0.2),
    }
    out = kernel(**ins)
    print("out", out.shape, out.dtype, float(np.abs(out).max()))



# revision 3
# speedup vs baseline: 1.1576x; 1.1576x over previous
"""MultiHeadDiffAttention kernel for 8 trn2 NeuronCores.

Sharding: tensor-parallel over heads (H=8, one head per core).
Per core (head h), per batch:
  qT/kT/vT = W @ x.T   [128 feat, 2048 tok]  (ec-outer accumulation, bf16)
  v [k, dh] via 16 PE transposes of vT
  scoresT[k, q] per diff-branch via row-packed PE matmuls (c=64, concurrent)
  exp on ScalarE, both branches in one [128,1024] ACTIVATE (scores ~ N(0,1):
  no max subtraction needed)
  denominators: ee pairs pre-summed on DVE, then ones-stationary matmuls
  (half the PE streams of per-kt ones-matmuls)
  uT = v-stationary matmul streaming exp at n=512
  per-q-block tail (recip -> broadcast -> combine) is software-pipelined into
  the NEXT q-block (kt==2 slot) so the PE never drains between q-blocks and
  the HAM clock gate stays warm.
Token ownership is mixed-batch: core c owns tokens [c*256,(c+1)*256) of BOTH
batches, so each AllToAll carries only real data (no zero padding) and
phase 3 splits into two halves, the first hiding under the second AllToAll.
Post-A2A the joint-head RMS reduction uses the ones-matmul trick; norm_w and
(1-dw) are folded into Wo on the host; Wo consumes oT directly.
"""

import os
import sys

import numpy as np

if "/opt/trn_rl_repo" not in sys.path:
    sys.path.insert(0, "/opt/trn_rl_repo")

B, S, E, H = 2, 2048, 1024, 8
DH = E // H          # 128
F = DH // 2          # 64
P = 128              # partitions
NCORES = 8
OWN = 256            # tokens owned per core per batch
EC = E // P          # 8 e-chunks
KC = S // P          # 16 k-chunks per batch
QBS = 512            # q-block size
QB = S // QBS        # 4 q-blocks per batch
LAG = 3              # consume lag (k-chunks) in the attention pipeline
EPS = float(np.finfo(np.float32).eps)

LAST_RESULTS = None  # BassKernelResults of the most recent run (test.py reads this)

_NC_CACHE: dict = {}


def _build(dw: float):
    import concourse.bass as bass
    import concourse.mybir as mybir
    import concourse.tile as tile
    from concourse import bacc
    from concourse.masks import make_identity

    dt = mybir.dt
    AF = mybir.ActivationFunctionType

    nc = bacc.Bacc("TRN2", target_bir_lowering=False, debug=False, num_devices=NCORES)

    xT_d = nc.dram_tensor("xT", [B, E, S], dt.bfloat16, kind="ExternalInput")
    wqT_d = nc.dram_tensor("wqT", [E, DH], dt.bfloat16, kind="ExternalInput")
    wkT_d = nc.dram_tensor("wkT", [E, DH], dt.bfloat16, kind="ExternalInput")
    wvT_d = nc.dram_tensor("wvT", [E, DH], dt.bfloat16, kind="ExternalInput")
    woT_d = nc.dram_tensor("woT", [E, E], dt.bfloat16, kind="ExternalInput")
    out_d = nc.dram_tensor("out", [B, OWN, E], dt.float32, kind="ExternalOutput")

    with tile.TileContext(nc) as tc:
        with (
            tc.tile_pool(name="consts", bufs=1) as consts,
            tc.tile_pool(name="xt", bufs=1) as xtp,
            tc.tile_pool(name="proj", bufs=1) as projp,
            tc.tile_pool(name="vv", bufs=2) as vvp,
            tc.tile_pool(name="expp", bufs=6) as expp,
            tc.tile_pool(name="esum", bufs=3) as esump,
            tc.tile_pool(name="small", bufs=2) as small,
            tc.tile_pool(name="mid", bufs=2) as mid,
            tc.tile_pool(name="osb", bufs=2) as osb,
            tc.tile_pool(name="p3", bufs=1) as p3,
            tc.tile_pool(name="dram", bufs=1, space="DRAM") as dram,
        ):
            eps_t = consts.tile([P, 1], dt.float32, tag="eps")
            nc.vector.memset(eps_t, EPS)
            ones_col = consts.tile([P, 32], dt.bfloat16, tag="ones_col")
            nc.vector.memset(ones_col, 1.0)
            ones_c1 = consts.tile([1, P], dt.float32, tag="ones_c1")
            nc.vector.memset(ones_c1, 1.0)
            negdw_c1 = consts.tile([1, P], dt.float32, tag="negdw_c1")
            nc.vector.memset(negdw_c1, -dw)
            ident = consts.tile([P, P], dt.bfloat16, tag="ident")
            make_identity(nc, ident)
            # prime the exp table set during the initial DMA wait
            scratch = consts.tile([P, 32], dt.bfloat16, tag="scratch")
            nc.scalar.activation(scratch, ones_col, AF.Exp)

            wq_sb = consts.tile([P, EC, DH], dt.bfloat16, tag="wq")
            wk_sb = consts.tile([P, EC, DH], dt.bfloat16, tag="wk")
            wv_sb = consts.tile([P, EC, DH], dt.bfloat16, tag="wv")
            for w_sb, w_d in ((wq_sb, wqT_d), (wk_sb, wkT_d), (wv_sb, wvT_d)):
                nc.sync.dma_start(
                    out=w_sb, in_=w_d.rearrange("(c p) d -> p c d", p=P)
                )
            # both batches of xT up front, chunk-wise (compute starts on chunk 0)
            xts = []
            for b in range(B):
                xt = xtp.tile([P, EC, S], dt.bfloat16, tag=f"xt{b}", name=f"xt{b}")
                xT_v = xT_d[b].rearrange("(c p) t -> c p t", p=P)
                for ec in range(EC):
                    nc.sync.dma_start(out=xt[:, ec, :], in_=xT_v[ec])
                xts.append(xt)
            wo_sb = consts.tile([P, EC, E], dt.bfloat16, tag="wo")
            nc.sync.dma_start(out=wo_sb, in_=woT_d.rearrange("(c p) e -> p c e", p=P))

            # Per-batch AllToAll buffers: block d = head-h output for tokens
            # [d*OWN, (d+1)*OWN) of this batch. No padding.
            a2a_in = [
                dram.tile([NCORES, DH, OWN], dt.bfloat16, tag=f"a2a_in{b}",
                          name=f"a2a_in{b}")
                for b in range(B)
            ]
            a2a_out = [
                dram.tile([NCORES, DH, OWN], dt.bfloat16, tag=f"a2a_out{b}",
                          name=f"a2a_out{b}")
                for b in range(B)
            ]

            oTb = [None, None]   # phase-3 [P, H, OWN] per half
            ssq_sb = [None, None]

            with (
                tc.tile_pool(name="psA", bufs=2, space="PSUM") as psA,
                tc.tile_pool(name="psU", bufs=1, space="PSUM") as psU,
                tc.tile_pool(name="psS", bufs=2, space="PSUM") as psS,
            ):
                def stats_half(hb):
                    # sum of squares over all heads for my OWN tokens of batch hb
                    sq = p3.tile([P, H, OWN], dt.bfloat16, tag="sq", bufs=2)
                    nc.scalar.activation(sq, oTb[hb], AF.Square)
                    # borrow a scores-pool slot (the ds slots are mid-
                    # accumulation here, sc slots rotate quickly)
                    ssq = psA.tile([P, 2, QBS], dt.float32, tag="sc", name="ssq")
                    for fc in range(EC):
                        nc.tensor.matmul(
                            ssq[0:32, 0, :OWN], lhsT=ones_col, rhs=sq[:, fc, :],
                            start=(fc == 0), stop=(fc == EC - 1),
                        )
                    srow = p3.tile([1, OWN], dt.float32, tag=f"ssq{hb}",
                                   name=f"ssq{hb}")
                    nc.vector.tensor_copy(srow, ssq[0:32, 0, :OWN][0:1, :])
                    ssq_sb[hb] = srow

                for b in range(B):
                    xt = xts[b]
                    # --- projections: ec-outer accumulation, 2 PSUM tiles ---
                    qT = projp.tile([P, S], dt.bfloat16, tag="qT", name="qT")
                    kT = projp.tile([P, S], dt.bfloat16, tag="kT", name="kT")
                    vTs = projp.tile([P, S], dt.bfloat16, tag="vTs", name="vTs")
                    for w_sb, dst in ((wq_sb, qT), (wk_sb, kT), (wv_sb, vTs)):
                        ps0 = psA.tile([P, 2, QBS], dt.float32, tag="sc", name="ps0")
                        ps1 = psA.tile([P, 2, QBS], dt.float32, tag="sc", name="ps1")
                        pss = (ps0, ps1)
                        for ec in range(EC):
                            for tb in range(4):
                                nc.tensor.matmul(
                                    pss[tb // 2][:, tb % 2, :],
                                    lhsT=w_sb[:, ec, :],
                                    rhs=xt[:, ec, tb * QBS:(tb + 1) * QBS],
                                    start=(ec == 0),
                                    stop=(ec == EC - 1),
                                )
                        for tb in range(4):
                            nc.vector.tensor_copy(
                                dst[:, tb * QBS:(tb + 1) * QBS],
                                pss[tb // 2][:, tb % 2, :],
                            )
                    # v [k-tok, dh] via PE transposes of vT
                    v = vvp.tile([P, KC, DH], dt.bfloat16, tag="v", name="v")
                    for gr in range(2):
                        tp = psU.tile([P, 8, P], dt.bfloat16, tag="u12", name="tp")
                        for j in range(8):
                            kt = gr * 8 + j
                            nc.tensor.transpose(
                                tp[:, j, :], vTs[:, kt * P:(kt + 1) * P], ident
                            )
                        nc.vector.tensor_copy(v[:, gr * 8:(gr + 1) * 8, :], tp)

                    # --- attention ---
                    pending = None

                    def emit_tail():
                        nonlocal pending
                        pb, pqb, rrow, u12p = pending
                        pending = None
                        rps = psA.tile([P, 2, QBS], dt.float32, tag="sc", name="rps")
                        nc.tensor.matmul(rps[:, 0, :], lhsT=ones_c1,
                                         rhs=rrow[:, 0, :])
                        nc.tensor.matmul(rps[:, 1, :], lhsT=negdw_c1,
                                         rhs=rrow[:, 1, :])
                        rr = mid.tile([P, 2, QBS], dt.float32, tag="rr", name="rr")
                        nc.vector.tensor_copy(rr, rps)
                        t1 = mid.tile([P, QBS], dt.float32, tag="t1", name="t1")
                        nc.vector.tensor_mul(t1, u12p[:, 0, :], rr[:, 0, :])
                        t2 = mid.tile([P, QBS], dt.float32, tag="t2", name="t2")
                        nc.vector.tensor_mul(t2, u12p[:, 1, :], rr[:, 1, :])
                        oT = osb.tile([P, QBS], dt.bfloat16, tag="oT", name="oT")
                        nc.vector.tensor_add(oT, t1, t2)
                        for half in range(2):
                            nc.sync.dma_start(
                                out=a2a_in[pb][2 * pqb + half],
                                in_=oT[:, half * OWN:(half + 1) * OWN],
                            )

                    for qb in range(QB):
                        qs = slice(qb * QBS, (qb + 1) * QBS)
                        u12 = psU.tile([P, 2, QBS], dt.float32, tag="u12",
                                       name="u12")
                        ds1 = psS.tile([32, QBS], dt.float32, tag="ds", name="ds1")
                        ds2 = psS.tile([32, QBS], dt.float32, tag="ds", name="ds2")
                        ees = []
                        ess = []

                        def consume(kt):
                            if kt % 2 == 1:
                                pr = kt // 2
                                es = ess[pr]
                                nc.tensor.matmul(
                                    ds1, lhsT=ones_col, rhs=es[:, 0, :],
                                    start=(pr == 0), stop=(pr == KC // 2 - 1),
                                )
                                nc.tensor.matmul(
                                    ds2, lhsT=ones_col, rhs=es[:, 1, :],
                                    start=(pr == 0), stop=(pr == KC // 2 - 1),
                                )
                            ee = ees[kt]
                            nc.tensor.matmul(
                                u12[:, 0, :], lhsT=v[:, kt, :], rhs=ee[:, 0, :],
                                start=(kt == 0), stop=(kt == KC - 1),
                            )
                            nc.tensor.matmul(
                                u12[:, 1, :], lhsT=v[:, kt, :], rhs=ee[:, 1, :],
                                start=(kt == 0), stop=(kt == KC - 1),
                            )

                        for kt in range(KC):
                            ks = slice(kt * P, (kt + 1) * P)
                            s12 = psA.tile([P, 2, QBS], dt.float32, tag="sc",
                                           name="s12")
                            nc.tensor.matmul(s12[:, 0, :], lhsT=kT[0:F, ks],
                                             rhs=qT[0:F, qs])
                            nc.tensor.matmul(s12[:, 1, :], lhsT=kT[F:P, ks],
                                             rhs=qT[F:P, qs])
                            ee = expp.tile([P, 2, QBS], dt.bfloat16, tag="ee",
                                           name="ee")
                            nc.scalar.activation(ee, s12, AF.Exp, scale=F**-0.5)
                            ees.append(ee)
                            if kt % 2 == 1:
                                es = esump.tile([P, 2, QBS], dt.bfloat16,
                                                tag="es", name="es")
                                nc.vector.tensor_add(es, ees[kt - 1], ee)
                                ess.append(es)
                            if kt == 2 and pending is not None:
                                emit_tail()
                            if kt == 8 and b == 1 and qb == 1:
                                # sum-of-squares for phase-3 half 0, hidden here
                                stats_half(0)
                            if kt >= LAG:
                                consume(kt - LAG)
                        for kt in range(KC - LAG, KC):
                            consume(kt)

                        rrow = small.tile([1, 2, QBS], dt.float32, tag="rrow",
                                          name="rrow")
                        nc.vector.reciprocal_approx_fast(rrow[:, 0, :],
                                                         ds1[0:1, :])
                        nc.vector.reciprocal_approx_fast(rrow[:, 1, :],
                                                         ds2[0:1, :])
                        pending = (b, qb, rrow, u12)
                        if qb == QB - 1:
                            emit_tail()

                    nc.gpsimd.collective_compute(
                        "AllToAll",
                        mybir.AluOpType.bypass,
                        replica_groups=[list(range(NCORES))],
                        ins=[a2a_in[b].opt()],
                        outs=[a2a_out[b].opt()],
                    )
                    # prefetch my joint-head slice for phase 3
                    oTb[b] = p3.tile([P, H, OWN], dt.bfloat16, tag=f"oTb{b}",
                                     name=f"oTb{b}")
                    nc.sync.dma_start(
                        out=oTb[b], in_=a2a_out[b].rearrange("h p t -> p h t")
                    )

            # --- phase 3: RMS norm + output projection, per 256-token half ---
            with (
                tc.tile_pool(name="p3sc", bufs=1, space="PSUM") as p3sc,
                tc.tile_pool(name="p3wo", bufs=2, space="PSUM") as p3wo,
                tc.tile_pool(name="p3ss", bufs=2, space="PSUM") as p3ss,
            ):
                def stats_half_late(hb):
                    sq = p3.tile([P, H, OWN], dt.bfloat16, tag="sq", bufs=2)
                    nc.scalar.activation(sq, oTb[hb], AF.Square)
                    ssq = p3ss.tile([32, OWN], dt.float32, tag="ssq")
                    for fc in range(EC):
                        nc.tensor.matmul(
                            ssq, lhsT=ones_col, rhs=sq[:, fc, :],
                            start=(fc == 0), stop=(fc == EC - 1),
                        )
                    srow = p3.tile([1, OWN], dt.float32, tag=f"ssq{hb}",
                                   name=f"ssq{hb}")
                    nc.vector.tensor_copy(srow, ssq[0:1, :])
                    ssq_sb[hb] = srow

                for hb in range(B):
                    if ssq_sb[hb] is None:
                        stats_half_late(hb)
                    sroot = small.tile([1, OWN], dt.float32, tag="sroot",
                                       name="sroot")
                    nc.scalar.activation(
                        sroot, ssq_sb[hb], AF.Sqrt, scale=1.0 / E,
                        bias=eps_t[0:1, :]
                    )
                    rmsrow = small.tile([1, OWN], dt.float32, tag="rmsrow",
                                        name="rmsrow")
                    nc.vector.reciprocal_approx_fast(rmsrow, sroot)
                    rmsps = p3sc.tile([P, OWN], dt.float32, tag="rps", name="rmsps")
                    nc.tensor.matmul(rmsps, lhsT=ones_c1, rhs=rmsrow)
                    rmsb = p3.tile([P, OWN], dt.bfloat16, tag="rmsb", bufs=2,
                                   name="rmsb")
                    nc.vector.tensor_copy(rmsb, rmsps)

                    nrm = p3.tile([P, H, OWN], dt.bfloat16, tag="nrm", bufs=2,
                                  name="nrm")
                    nc.vector.tensor_mul(
                        nrm, oTb[hb],
                        rmsb[:, None, :].broadcast_to([P, H, OWN]),
                    )

                    out_v = out_d[hb].rearrange("(t p) e -> t p e", p=P)
                    for tt in range(OWN // P):
                        wops = p3wo.tile([P, 2, QBS], dt.float32, tag="wo",
                                         name="wops")
                        for fc in range(EC):
                            for nb in range(2):
                                nc.tensor.matmul(
                                    wops[:, nb, :],
                                    lhsT=nrm[:, fc, tt * P:(tt + 1) * P],
                                    rhs=wo_sb[:, fc, nb * QBS:(nb + 1) * QBS],
                                    start=(fc == 0),
                                    stop=(fc == EC - 1),
                                )
                        out_sb = p3.tile([P, E], dt.float32, tag="out_sb", bufs=2,
                                         name="out_sb")
                        nc.vector.tensor_copy(
                            out_sb.rearrange("p (n q) -> p n q", n=2), wops
                        )
                        nc.sync.dma_start(out=out_v[tt], in_=out_sb)

    nc.compile()
    return nc


def _get_nc(dw: float):
    key = round(float(dw), 9)
    if key not in _NC_CACHE:
        _NC_CACHE[key] = _build(float(dw))
    return _NC_CACHE[key]


def kernel(x, Wq, Wk, Wv, norm_w, Wo, bo, diff_weight):
    import ml_dtypes

    from concourse.bass_utils import run_bass_kernel_spmd

    global LAST_RESULTS

    bf16 = ml_dtypes.bfloat16
    x = np.asarray(x, dtype=np.float32)
    Wq = np.asarray(Wq, dtype=np.float32)
    Wk = np.asarray(Wk, dtype=np.float32)
    Wv = np.asarray(Wv, dtype=np.float32)
    Wo = np.asarray(Wo, dtype=np.float32)
    norm_w = np.asarray(norm_w, dtype=np.float32)
    bo = np.asarray(bo, dtype=np.float32)
    dw = float(np.asarray(diff_weight))

    nc = _get_nc(dw)

    xT = np.ascontiguousarray(x.transpose(0, 2, 1)).astype(bf16)  # [B, E, S]
    woT = np.ascontiguousarray(
        (Wo * norm_w.reshape(-1)[None, :] * (1.0 - dw)).T
    ).astype(bf16)  # [E(feat), E(out)]

    in_maps = []
    for h in range(NCORES):
        rows = slice(h * DH, (h + 1) * DH)
        in_maps.append(
            {
                "xT": xT,
                "wqT": np.ascontiguousarray(Wq[rows, :].T).astype(bf16),
                "wkT": np.ascontiguousarray(Wk[rows, :].T).astype(bf16),
                "wvT": np.ascontiguousarray(Wv[rows, :].T).astype(bf16),
                "woT": woT,
            }
        )

    res = run_bass_kernel_spmd(
        nc,
        in_maps,
        core_ids=list(range(NCORES)),
        trace=bool(os.environ.get("KERNEL_TRACE")),
    )
    LAST_RESULTS = res

    full = np.empty((B, S, E), dtype=np.float32)
    for c in range(NCORES):
        o = res.results[c]["out"]  # [B, OWN, E]
        for b in range(B):
            full[b, c * OWN:(c + 1) * OWN, :] = o[b]
    full = full + (1.0 - dw) * bo[None, None, :]
    return full


if __name__ == "__main__":
    rng = np.random.default_rng(0)
    sc = E**-0.5
    ins = {
        "x": rng.standard_normal((B, S, E), dtype=np.float32),
        "Wq": rng.standard_normal((E, E), dtype=np.float32) * sc,
        "Wk": rng.standard_normal((E, E), dtype=np.float32) * sc,
        "Wv": rng.standard_normal((E, E), dtype=np.float32) * sc,
        "norm_w": np.ones((H, DH), dtype=np.float32),
        "Wo": rng.standard_normal((E, E), dtype=np.float32) * sc,
        "bo": np.zeros((E,), dtype=np.float32),
        "diff_weight": np.float32(0.2),
    }
    out = kernel(**ins)
    print("out", out.shape, out.dtype, float(np.abs(out).max()))
